# revision 3
# baseline (speedup 1.0000x reference)
"""Binary-weight 3x3 conv2d (stride 1, pad 1) on 8 TRN2 NeuronCores.

Reference computes y = conv2d(x, sign(weights)) in NCHW/OIHW, f32.
  x: (32, 128, 56, 56) f32, weights: (256, 128, 3, 3) f32 -> y: (32, 256, 56, 56) f32

Strategy (data-parallel on batch: 8 cores x 4 images; weights replicated):
  - Host side: x is split as x = hi + lo with hi = e4m3(x), lo = e4m3(x - hi)
    (recon err 7.5e-4), pre-padded to 58x58; weights binarized to +-1 fp8.
  - Each conv tap becomes ONE fp8 DoubleRow matmul: the PE multiplies the
    (hi, lo) slot pair by the same +-1 weight in both rows and sums - full
    9-tap conv = 9 DR matmuls per 8-row psum tile at ~2x fp8 throughput,
    ~0.57x the cycles of the bf16 kernel, with near-bf16 accuracy.
  - Weight-stationary ordering: for each (row-tile, co-half), each tap's
    weights are held across the 4 images' matmuls so the 256-column
    DoubleRow LDWEIGHTS hides behind 4 back-to-back matmuls. 4 PSUM banks
    accumulate while 4 drain.
  - Output stored as bf16 (halves store traffic), upcast to f32 on host.
    Total output rel err ~1.8e-3 (vs bf16 kernel's 1.66e-3).
"""

import time

import numpy as np
import ml_dtypes

import concourse.bass as bass
import concourse.bacc as bacc
import concourse.mybir as mybir
import concourse.tile as tile
from concourse.bass_utils import run_bass_kernel_spmd

N_CORES = 8
B, CI, H, W = 32, 128, 56, 56
CO = 256
KH = KW = 3
BPC = B // N_CORES          # images per core
HP, WP = H + 2, W + 2       # padded spatial
ROWS_PER_TILE = 8
N_ROW_TILES = H // ROWS_PER_TILE   # 7
NFREE = ROWS_PER_TILE * W          # 448 <= 512 (one PSUM bank)

F32 = mybir.dt.float32
BF16 = mybir.dt.bfloat16
F8 = mybir.dt.float8e4
E4NP = ml_dtypes.float8_e4m3

DEFAULT_CFG = dict(
    img_bufs=2,                 # rotation depth per image tile (prefetch next pass)
    out_bufs=8,
    psum_bufs=8,
    copy_engine="alternate",    # psum->sbuf drain engine
)


def _emit_body(nc, img_pool, out_pool, psum_pool, x8_d, y_d, w_sb, cfg):
    """One full pass: conv of the core's BPC images."""
    DR = mybir.MatmulPerfMode.DoubleRow
    imgs = []
    for n in range(BPC):
        xi = img_pool.tile([CI, 2, HP, WP], F8, name=f"img{n}")
        nc.sync.dma_start(xi[:], x8_d[n])
        imgs.append(xi)
    copy_i = 0
    for t in range(N_ROW_TILES):
        r0 = t * ROWS_PER_TILE
        for h in range(CO // 128):
            pss = [psum_pool.tile([128, NFREE], F32, name="ps") for _ in range(BPC)]
            for k9 in range(KH * KW):
                kh, kw = divmod(k9, KW)
                lhsT = w_sb[:, k9, :, h * 128:(h + 1) * 128]
                for n in range(BPC):
                    rhs = imgs[n][:, :, r0 + kh: r0 + kh + ROWS_PER_TILE,
                                  kw: kw + W]
                    nc.tensor.matmul(
                        pss[n], lhsT, rhs,
                        start=(k9 == 0), stop=(k9 == KH * KW - 1),
                        perf_mode=DR,
                    )
            for n in range(BPC):
                ob = out_pool.tile([128, NFREE], BF16, name="ob")
                ce = cfg["copy_engine"]
                if ce == "alternate":
                    ce = "vector" if copy_i % 2 == 0 else "scalar"
                copy_i += 1
                if ce == "vector":
                    nc.vector.tensor_copy(ob[:], pss[n][:])
                else:
                    nc.scalar.copy(ob[:], pss[n][:])
                nc.sync.dma_start(
                    y_d[n, h * 128:(h + 1) * 128, r0: r0 + ROWS_PER_TILE, :],
                    ob[:],
                )


def build_program(static_reps: int = 1, **overrides) -> bass.Bass:
    cfg = dict(DEFAULT_CFG, **overrides)
    nc = bacc.Bacc(name="binconv2d")
    x8_d = nc.dram_tensor("x8", (BPC, CI, 2, HP, WP), F8, kind="ExternalInput")
    w8_d = nc.dram_tensor("w8", (KH * KW, CI, 2, CO), F8, kind="ExternalInput")
    y_d = nc.dram_tensor("y", (BPC, CO, H, W), BF16, kind="ExternalOutput")

    with tile.TileContext(nc) as tc:
        with (
            tc.tile_pool(name="wpool", bufs=1) as wpool,
            tc.tile_pool(name="imgs", bufs=cfg["img_bufs"]) as img_pool,
            tc.tile_pool(name="outb", bufs=cfg["out_bufs"]) as out_pool,
            tc.tile_pool(name="psum", bufs=cfg["psum_bufs"], space="PSUM") as psum_pool,
        ):
            w_sb = wpool.tile([CI, KH * KW, 2, CO], F8)
            nc.sync.dma_start(w_sb[:], w8_d[:].rearrange("k p s c -> p k s c"))
            for _ in range(static_reps):
                _emit_body(nc, img_pool, out_pool, psum_pool, x8_d, y_d, w_sb, cfg)

    nc.finalize()
    return nc


def prep_weights(weights: np.ndarray) -> np.ndarray:
    """sign(weights) as fp8 e4m3, laid out [kh*kw, ci, slot=2, co]."""
    bw = np.sign(np.asarray(weights, dtype=np.float32))
    # (co, ci, kh, kw) -> (kh*kw, ci, co)
    bw = np.ascontiguousarray(bw.transpose(2, 3, 1, 0)).reshape(KH * KW, CI, CO)
    w8 = np.empty((KH * KW, CI, 2, CO), dtype=E4NP)
    w8[:, :, 0, :] = bw.astype(E4NP)
    w8[:, :, 1, :] = w8[:, :, 0, :]
    return w8


def prep_x(x: np.ndarray) -> np.ndarray:
    """x -> pre-padded (B, CI, slot=2, 58, 58) e4m3 hi/lo split."""
    x = np.asarray(x, dtype=np.float32)
    hi = x.astype(E4NP)
    lo = (x - hi.astype(np.float32)).astype(E4NP)
    xp = np.zeros((B, CI, 2, HP, WP), dtype=E4NP)
    xp[:, :, 0, 1:H + 1, 1:W + 1] = hi
    xp[:, :, 1, 1:H + 1, 1:W + 1] = lo
    return xp


def make_in_maps(x: np.ndarray, weights: np.ndarray) -> list[dict]:
    xp = prep_x(x)
    w8 = prep_weights(weights)
    return [
        {"x8": xp[i * BPC:(i + 1) * BPC], "w8": w8}
        for i in range(N_CORES)
    ]


def _run_once(x, weights) -> np.ndarray:
    nc = build_program()
    in_maps = make_in_maps(x, weights)
    res = run_bass_kernel_spmd(nc, in_maps, core_ids=list(range(N_CORES)))
    yb = np.concatenate([r["y"] for r in res.results], axis=0)
    return yb.astype(np.float32)


_SUBPROC_SRC = """
import sys, numpy as np
sys.path.insert(0, sys.argv[1])
import kernel as K
x = np.load(sys.argv[2]); w = np.load(sys.argv[3])
np.save(sys.argv[4], K._run_once(x, w))
"""


def kernel(x, weights) -> np.ndarray:
    x = np.ascontiguousarray(np.asarray(x, dtype=np.float32))
    weights = np.ascontiguousarray(np.asarray(weights, dtype=np.float32))
    try:
        return _run_once(x, weights)
    except Exception as first_exc:
        # Transient device wedges (NRT_EXEC_UNIT_UNRECOVERABLE, mesh desync)
        # poison the in-process PJRT client; only a fresh process recovers.
        import os
        import subprocess
        import sys
        import tempfile

        last_exc = first_exc
        moddir = os.path.dirname(os.path.abspath(__file__))
        for attempt in range(2):
            time.sleep(10 * (attempt + 1))
            try:
                with tempfile.TemporaryDirectory() as td:
                    xp, wp, yp = (os.path.join(td, f) for f in
                                  ("x.npy", "w.npy", "y.npy"))
                    np.save(xp, x)
                    np.save(wp, weights)
                    subprocess.run(
                        [sys.executable, "-c", _SUBPROC_SRC, moddir, xp, wp, yp],
                        check=True, timeout=900,
                    )
                    return np.load(yp)
            except Exception as e:
                last_exc = e
        raise last_exc


# revision 6
# speedup vs baseline: 1.0398x; 1.0398x over previous
"""Binary-weight 3x3 conv2d (stride 1, pad 1) on 8 TRN2 NeuronCores.

Reference computes y = conv2d(x, sign(weights)) in NCHW/OIHW, f32.
  x: (32, 128, 56, 56) f32, weights: (256, 128, 3, 3) f32 -> y: (32, 256, 56, 56) f32

Strategy (data-parallel on batch: 8 cores x 4 images; weights replicated):
  - Host side: x is split as x = hi + lo with hi = e4m3(x), lo = e4m3(x - hi)
    (recon err 7.5e-4), pre-padded to 58x58; weights binarized to +-1 fp8.
  - Each conv tap becomes ONE fp8 DoubleRow matmul: the PE multiplies the
    (hi, lo) slot pair by the same +-1 weight in both rows and sums - full
    9-tap conv = 9 DR matmuls per 8-row psum tile at ~2x fp8 throughput,
    ~0.57x the cycles of the bf16 kernel, with near-bf16 accuracy.
  - Weight-stationary ordering: for each (row-tile, co-half), each tap's
    weights are held across the 4 images' matmuls so the 256-column
    DoubleRow LDWEIGHTS hides behind 4 back-to-back matmuls. 4 PSUM banks
    accumulate while 4 drain.
  - Output stored as bf16 (halves store traffic), upcast to f32 on host.
    Total output rel err ~1.8e-3 (vs bf16 kernel's 1.66e-3).
"""

import time

import numpy as np
import ml_dtypes

import concourse.bass as bass
import concourse.bacc as bacc
import concourse.mybir as mybir
import concourse.tile as tile
from concourse.bass_utils import run_bass_kernel_spmd

N_CORES = 8
B, CI, H, W = 32, 128, 56, 56
CO = 256
KH = KW = 3
BPC = B // N_CORES          # images per core
HP, WP = H + 2, W + 2       # padded spatial
ROWS_PER_TILE = 8
N_ROW_TILES = H // ROWS_PER_TILE   # 7
NFREE = ROWS_PER_TILE * W          # 448 <= 512 (one PSUM bank)

F32 = mybir.dt.float32
BF16 = mybir.dt.bfloat16
F8 = mybir.dt.float8e4
E4NP = ml_dtypes.float8_e4m3

DEFAULT_CFG = dict(
    img_bufs=2,                 # rotation depth per image tile (prefetch next pass)
    out_bufs=8,
    psum_bufs=8,
    copy_engine="alternate",    # psum->sbuf drain engine
    store_queue="sync",         # DMA queue for output stores: sync|scalar|split
    load_queue="sync",          # DMA queue for image loads
    skip_store=False,           # ablation: only store on first pass
    load_once=False,            # ablation: load images once outside rep loop
)


def _load_imgs(nc, img_pool, x8_d, cfg):
    q = {"sync": nc.sync, "scalar": nc.scalar, "vector": nc.vector}[cfg["load_queue"]]
    imgs = []
    for n in range(BPC):
        xi = img_pool.tile([CI, 2, HP, WP], F8, name=f"img{n}")
        q.dma_start(xi[:], x8_d[n])
        imgs.append(xi)
    return imgs


def _emit_body(nc, img_pool, out_pool, psum_pool, x8_d, y_d, w_sb, cfg,
               state, imgs=None):
    """One full pass: conv of the core's BPC images."""
    DR = mybir.MatmulPerfMode.DoubleRow
    if imgs is None:
        imgs = _load_imgs(nc, img_pool, x8_d, cfg)
    pass_i = state["pass"]
    state["pass"] += 1
    copy_i = 0
    for t in range(N_ROW_TILES):
        r0 = t * ROWS_PER_TILE
        for h in range(CO // 128):
            pss = [psum_pool.tile([128, NFREE], F32, name="ps") for _ in range(BPC)]
            for k9 in range(KH * KW):
                kh, kw = divmod(k9, KW)
                lhsT = w_sb[:, k9, :, h * 128:(h + 1) * 128]
                for n in range(BPC):
                    rhs = imgs[n][:, :, r0 + kh: r0 + kh + ROWS_PER_TILE,
                                  kw: kw + W]
                    nc.tensor.matmul(
                        pss[n], lhsT, rhs,
                        start=(k9 == 0), stop=(k9 == KH * KW - 1),
                        perf_mode=DR,
                    )
            for n in range(BPC):
                ob = out_pool.tile([128, NFREE], BF16, name="ob")
                ce = cfg["copy_engine"]
                if ce == "alternate":
                    ce = "vector" if copy_i % 2 == 0 else "scalar"
                if ce == "vector":
                    nc.vector.tensor_copy(ob[:], pss[n][:])
                else:
                    nc.scalar.copy(ob[:], pss[n][:])
                if not (cfg["skip_store"] and pass_i > 0):
                    sq = cfg["store_queue"]
                    if sq == "split":
                        sq = "sync" if copy_i % 2 == 0 else "scalar"
                    q = {"sync": nc.sync, "scalar": nc.scalar,
                         "vector": nc.vector}[sq]
                    q.dma_start(
                        y_d[n, h * 128:(h + 1) * 128, r0: r0 + ROWS_PER_TILE, :],
                        ob[:],
                    )
                copy_i += 1


def build_program(static_reps: int = 1, **overrides) -> bass.Bass:
    cfg = dict(DEFAULT_CFG, **overrides)
    nc = bacc.Bacc(name="binconv2d")
    x8_d = nc.dram_tensor("x8", (BPC, CI, 2, HP, WP), F8, kind="ExternalInput")
    w8_d = nc.dram_tensor("w8", (KH * KW, CI, 2, CO), F8, kind="ExternalInput")
    y_d = nc.dram_tensor("y", (BPC, CO, H, W), BF16, kind="ExternalOutput")

    with tile.TileContext(nc) as tc:
        with (
            tc.tile_pool(name="wpool", bufs=1) as wpool,
            tc.tile_pool(name="imgs", bufs=cfg["img_bufs"]) as img_pool,
            tc.tile_pool(name="outb", bufs=cfg["out_bufs"]) as out_pool,
            tc.tile_pool(name="psum", bufs=cfg["psum_bufs"], space="PSUM") as psum_pool,
        ):
            w_sb = wpool.tile([CI, KH * KW, 2, CO], F8)
            nc.sync.dma_start(w_sb[:], w8_d[:].rearrange("k p s c -> p k s c"))
            state = {"pass": 0}
            imgs = None
            if cfg["load_once"]:
                imgs = _load_imgs(nc, img_pool, x8_d, cfg)
            for _ in range(static_reps):
                _emit_body(nc, img_pool, out_pool, psum_pool, x8_d, y_d, w_sb,
                           cfg, state, imgs)

    nc.finalize()
    return nc


def prep_weights(weights: np.ndarray) -> np.ndarray:
    """sign(weights) as fp8 e4m3, laid out [kh*kw, ci, slot=2, co]."""
    bw = np.sign(np.asarray(weights, dtype=np.float32))
    # (co, ci, kh, kw) -> (kh*kw, ci, co)
    bw = np.ascontiguousarray(bw.transpose(2, 3, 1, 0)).reshape(KH * KW, CI, CO)
    w8 = np.empty((KH * KW, CI, 2, CO), dtype=E4NP)
    w8[:, :, 0, :] = bw.astype(E4NP)
    w8[:, :, 1, :] = w8[:, :, 0, :]
    return w8


def prep_x(x: np.ndarray) -> np.ndarray:
    """x -> pre-padded (B, CI, slot=2, 58, 58) e4m3 hi/lo split."""
    x = np.asarray(x, dtype=np.float32)
    hi = x.astype(E4NP)
    lo = (x - hi.astype(np.float32)).astype(E4NP)
    xp = np.zeros((B, CI, 2, HP, WP), dtype=E4NP)
    xp[:, :, 0, 1:H + 1, 1:W + 1] = hi
    xp[:, :, 1, 1:H + 1, 1:W + 1] = lo
    return xp


def make_in_maps(x: np.ndarray, weights: np.ndarray) -> list[dict]:
    xp = prep_x(x)
    w8 = prep_weights(weights)
    return [
        {"x8": xp[i * BPC:(i + 1) * BPC], "w8": w8}
        for i in range(N_CORES)
    ]


def _run_once(x, weights) -> np.ndarray:
    nc = build_program()
    in_maps = make_in_maps(x, weights)
    res = run_bass_kernel_spmd(nc, in_maps, core_ids=list(range(N_CORES)))
    yb = np.concatenate([r["y"] for r in res.results], axis=0)
    return yb.astype(np.float32)


_SUBPROC_SRC = """
import sys, numpy as np
sys.path.insert(0, sys.argv[1])
import kernel as K
x = np.load(sys.argv[2]); w = np.load(sys.argv[3])
np.save(sys.argv[4], K._run_once(x, w))
"""


def kernel(x, weights) -> np.ndarray:
    x = np.ascontiguousarray(np.asarray(x, dtype=np.float32))
    weights = np.ascontiguousarray(np.asarray(weights, dtype=np.float32))
    try:
        return _run_once(x, weights)
    except Exception as first_exc:
        # Transient device wedges (NRT_EXEC_UNIT_UNRECOVERABLE, mesh desync)
        # poison the in-process PJRT client; only a fresh process recovers.
        import os
        import subprocess
        import sys
        import tempfile

        last_exc = first_exc
        moddir = os.path.dirname(os.path.abspath(__file__))
        for attempt in range(2):
            time.sleep(10 * (attempt + 1))
            try:
                with tempfile.TemporaryDirectory() as td:
                    xp, wp, yp = (os.path.join(td, f) for f in
                                  ("x.npy", "w.npy", "y.npy"))
                    np.save(xp, x)
                    np.save(wp, weights)
                    subprocess.run(
                        [sys.executable, "-c", _SUBPROC_SRC, moddir, xp, wp, yp],
                        check=True, timeout=900,
                    )
                    return np.load(yp)
            except Exception as e:
                last_exc = e
        raise last_exc


# revision 8
# speedup vs baseline: 2.6280x; 2.5275x over previous
"""Binary-weight 3x3 conv2d (stride 1, pad 1) on 8 TRN2 NeuronCores.

Reference computes y = conv2d(x, sign(weights)) in NCHW/OIHW, f32.
  x: (32, 128, 56, 56) f32, weights: (256, 128, 3, 3) f32 -> y: (32, 256, 56, 56) f32

Strategy (data-parallel on batch: 8 cores x 4 images; weights replicated):
  - All compute in fp8 e4m3 DoubleRow matmuls (2 multiplies/cell/cycle).
    Host splits x = hi + lo (hi = e4m3(x), lo = e4m3(x - hi)) and ships 3
    pre-padded 58x58 planes per image: (hi shifted down 2 rows, hi, lo).
  - Per 8-row psum tile, the 9 conv taps become 7 DR matmuls:
      2x "corner pair": slots (hi_down2, hi) = taps (2,kw)&(0,kw), kw in {0,2}
         -> these 4 taps carry plain e4m3 quantization error;
      5x "corrected": slots (hi, lo) same tap with the same +-1 weight in
         both slots -> exact to e4m3(lo residual) precision.
    Output rel err 1.77e-2 (limit 2e-2), measured bit-exact vs numpy model.
  - rhs is a contiguous 464-wide span (8 padded rows x 58) so the matmul
    ifmap AP stays 3D ([p][2][464] - the ISA limit); tap alignment is done
    by a fixed psum dst offset (D=2); pad/wrap contributions land in psum
    columns 0,1 of each 58-block, which the drain copy skips.
  - Weight-stationary ordering: each tap's weights held across the 4
    images' matmuls so the 256-col DoubleRow LDWEIGHTS hides; 4 PSUM banks
    accumulate while 4 drain.
  - Output stored bf16 (halves store traffic), upcast to f32 on host.
"""

import time

import numpy as np
import ml_dtypes

import concourse.bass as bass
import concourse.bacc as bacc
import concourse.mybir as mybir
import concourse.tile as tile
from concourse.bass_utils import run_bass_kernel_spmd

N_CORES = 8
B, CI, H, W = 32, 128, 56, 56
CO = 256
KH = KW = 3
BPC = B // N_CORES          # images per core
HP, WP = H + 2, W + 2       # padded spatial
ROWS_PER_TILE = 8
N_ROW_TILES = H // ROWS_PER_TILE   # 7
NSPAN = ROWS_PER_TILE * WP         # 464 contiguous rhs span
NPS = NSPAN + 2                    # 466 psum cols (D=2 shift)
PLANE = HP * WP                    # 3364

F32 = mybir.dt.float32
BF16 = mybir.dt.bfloat16
F8 = mybir.dt.float8e4
E4NP = ml_dtypes.float8_e4m3

# Uncorrected vertical corner pairs per preset (taps (2,kw)&(0,kw)).
# 7mm: err 1.77e-2 | 8mm: 1.26e-2 | 9mm: 1.8e-3  (offline, bf16 out)
PRESET_PAIRS = {"7mm": (0, 2), "8mm": (0,), "9mm": ()}

DEFAULT_CFG = dict(
    preset="7mm",
    img_bufs=2,                 # rotation depth per image tile
    out_bufs=8,
    psum_bufs=8,
    copy_engine="alternate",    # psum->sbuf drain engine
    store_queue="sync",         # DMA queue for output stores: sync|scalar|split
    load_queue="sync",          # DMA queue for image loads
    skip_store=False,           # ablation: only store on first pass
    load_once=False,            # ablation: load images once outside rep loop
)


def _mm_descs(preset: str):
    """Per row-tile MM list: (wa_tap, wb_tap, slot0, kh_base, kw_base).
    rhs base = (r0 + kh_base)*WP + kw_base, slots [slot0:slot0+2], dst D=2.
    slot planes: 0 = hi shifted down 2 rows, 1 = hi, 2 = lo."""
    pair_kws = PRESET_PAIRS[preset]
    descs = []
    uncorrected = set()
    for kw in pair_kws:
        descs.append((2 * KW + kw, 0 * KW + kw, 0, 0, kw))
        uncorrected.update((2 * KW + kw, 0 * KW + kw))
    for k9 in range(KH * KW):
        if k9 in uncorrected:
            continue
        kh, kw = divmod(k9, KW)
        descs.append((k9, k9, 1, kh, kw))
    return descs


def _load_imgs(nc, img_pool, x3_d, cfg):
    q = {"sync": nc.sync, "scalar": nc.scalar, "vector": nc.vector}[cfg["load_queue"]]
    imgs = []
    for n in range(BPC):
        xi = img_pool.tile([CI, 3, PLANE], F8, name=f"img{n}")
        q.dma_start(xi[:], x3_d[n])
        imgs.append(xi)
    return imgs


def _emit_body(nc, img_pool, out_pool, psum_pool, x3_d, y_d, w_sb, cfg,
               state, imgs=None):
    """One full pass: conv of the core's BPC images."""
    DR = mybir.MatmulPerfMode.DoubleRow
    descs = _mm_descs(cfg["preset"])
    if imgs is None:
        imgs = _load_imgs(nc, img_pool, x3_d, cfg)
    pass_i = state["pass"]
    state["pass"] += 1
    copy_i = 0
    for t in range(N_ROW_TILES):
        r0 = t * ROWS_PER_TILE
        for h in range(CO // 128):
            pss = [psum_pool.tile([128, NPS], F32, name="ps") for _ in range(BPC)]
            for m, (_, _, s0, khb, kwb) in enumerate(descs):
                lhsT = w_sb[:, m, :, h * 128:(h + 1) * 128]
                b = (r0 + khb) * WP + kwb
                # tail clip: overhang past the plane maps to junk psum cols only
                span = min(NSPAN, PLANE - b)
                for n in range(BPC):
                    nc.tensor.matmul(
                        pss[n][:, 2:2 + span], lhsT,
                        imgs[n][:, s0:s0 + 2, b:b + span],
                        start=(m == 0), stop=(m == len(descs) - 1),
                        perf_mode=DR,
                    )
            for n in range(BPC):
                ob = out_pool.tile([128, ROWS_PER_TILE, W], BF16, name="ob")
                psv = pss[n][:, 2:2 + NSPAN].rearrange("p (r c) -> p r c", c=WP)
                ce = cfg["copy_engine"]
                if ce == "alternate":
                    ce = "vector" if copy_i % 2 == 0 else "scalar"
                if ce == "vector":
                    nc.vector.tensor_copy(ob[:], psv[:, :, 0:W])
                else:
                    nc.scalar.copy(ob[:], psv[:, :, 0:W])
                if not (cfg["skip_store"] and pass_i > 0):
                    sq = cfg["store_queue"]
                    if sq == "split":
                        sq = "sync" if copy_i % 2 == 0 else "scalar"
                    q = {"sync": nc.sync, "scalar": nc.scalar,
                         "vector": nc.vector}[sq]
                    q.dma_start(
                        y_d[n, h * 128:(h + 1) * 128, r0: r0 + ROWS_PER_TILE, :],
                        ob[:],
                    )
                copy_i += 1


def build_program(static_reps: int = 1, **overrides) -> bass.Bass:
    cfg = dict(DEFAULT_CFG, **overrides)
    n_mm = len(_mm_descs(cfg["preset"]))
    nc = bacc.Bacc(name="binconv2d")
    x3_d = nc.dram_tensor("x3", (BPC, CI, 3, PLANE), F8, kind="ExternalInput")
    w_d = nc.dram_tensor("w", (n_mm, CI, 2, CO), F8, kind="ExternalInput")
    y_d = nc.dram_tensor("y", (BPC, CO, H, W), BF16, kind="ExternalOutput")

    with tile.TileContext(nc) as tc:
        with (
            tc.tile_pool(name="wpool", bufs=1) as wpool,
            tc.tile_pool(name="imgs", bufs=cfg["img_bufs"]) as img_pool,
            tc.tile_pool(name="outb", bufs=cfg["out_bufs"]) as out_pool,
            tc.tile_pool(name="psum", bufs=cfg["psum_bufs"], space="PSUM") as psum_pool,
        ):
            w_sb = wpool.tile([CI, n_mm, 2, CO], F8)
            nc.sync.dma_start(w_sb[:], w_d[:].rearrange("m p s c -> p m s c"))
            state = {"pass": 0}
            imgs = None
            if cfg["load_once"]:
                imgs = _load_imgs(nc, img_pool, x3_d, cfg)
            for _ in range(static_reps):
                _emit_body(nc, img_pool, out_pool, psum_pool, x3_d, y_d, w_sb,
                           cfg, state, imgs)

    nc.finalize()
    return nc


def prep_weights(weights: np.ndarray, preset: str = None) -> np.ndarray:
    """sign(weights) as e4m3 laid out per-MM: [n_mm, ci, slot=2, co]."""
    preset = preset or DEFAULT_CFG["preset"]
    bw = np.sign(np.asarray(weights, dtype=np.float32))
    # (co, ci, kh, kw) -> (kh*kw, ci, co)
    bw = np.ascontiguousarray(bw.transpose(2, 3, 1, 0)).reshape(KH * KW, CI, CO)
    descs = _mm_descs(preset)
    w8 = np.zeros((len(descs), CI, 2, CO), dtype=E4NP)
    for m, (a, b, s0, khb, kwb) in enumerate(descs):
        w8[m, :, 0, :] = bw[a].astype(E4NP)
        w8[m, :, 1, :] = bw[b].astype(E4NP)
    return w8


def prep_x(x: np.ndarray) -> np.ndarray:
    """x -> (B, CI, plane=3, 58*58) e4m3: (hi down-shifted 2 rows, hi, lo)."""
    x = np.asarray(x, dtype=np.float32)
    hi = x.astype(E4NP)
    lo = (x - hi.astype(np.float32)).astype(E4NP)
    xp = np.zeros((B, CI, 3, HP, WP), dtype=E4NP)
    xp[:, :, 1, 1:H + 1, 1:W + 1] = hi
    xp[:, :, 2, 1:H + 1, 1:W + 1] = lo
    xp[:, :, 0, 0:HP - 2, :] = xp[:, :, 1, 2:HP, :]
    return xp.reshape(B, CI, 3, PLANE)


def make_in_maps(x: np.ndarray, weights: np.ndarray, preset: str = None) -> list[dict]:
    xp = prep_x(x)
    w8 = prep_weights(weights, preset)
    return [
        {"x3": xp[i * BPC:(i + 1) * BPC], "w": w8}
        for i in range(N_CORES)
    ]


def _run_once(x, weights) -> np.ndarray:
    nc = build_program()
    in_maps = make_in_maps(x, weights)
    res = run_bass_kernel_spmd(nc, in_maps, core_ids=list(range(N_CORES)))
    yb = np.concatenate([r["y"] for r in res.results], axis=0)
    return yb.astype(np.float32)


_SUBPROC_SRC = """
import sys, numpy as np
sys.path.insert(0, sys.argv[1])
import kernel as K
x = np.load(sys.argv[2]); w = np.load(sys.argv[3])
np.save(sys.argv[4], K._run_once(x, w))
"""


def kernel(x, weights) -> np.ndarray:
    x = np.ascontiguousarray(np.asarray(x, dtype=np.float32))
    weights = np.ascontiguousarray(np.asarray(weights, dtype=np.float32))
    try:
        return _run_once(x, weights)
    except Exception as first_exc:
        # Transient device wedges (NRT_EXEC_UNIT_UNRECOVERABLE, mesh desync)
        # poison the in-process PJRT client; only a fresh process recovers.
        import os
        import subprocess
        import sys
        import tempfile

        last_exc = first_exc
        moddir = os.path.dirname(os.path.abspath(__file__))
        for attempt in range(2):
            time.sleep(10 * (attempt + 1))
            try:
                with tempfile.TemporaryDirectory() as td:
                    xp, wp, yp = (os.path.join(td, f) for f in
                                  ("x.npy", "w.npy", "y.npy"))
                    np.save(xp, x)
                    np.save(wp, weights)
                    subprocess.run(
                        [sys.executable, "-c", _SUBPROC_SRC, moddir, xp, wp, yp],
                        check=True, timeout=900,
                    )
                    return np.load(yp)
            except Exception as e:
                last_exc = e
        raise last_exc


# revision 13
# speedup vs baseline: 3.3728x; 1.2834x over previous
"""Binary-weight 3x3 conv2d (stride 1, pad 1) on 8 TRN2 NeuronCores.

Reference computes y = conv2d(x, sign(weights)) in NCHW/OIHW, f32.
  x: (32, 128, 56, 56) f32, weights: (256, 128, 3, 3) f32 -> y: (32, 256, 56, 56) f32

Strategy (data-parallel on batch: 8 cores x 4 images; weights replicated):
  - All compute in fp8 e4m3 DoubleRow matmuls (2 multiplies/cell/cycle).
    Host splits x = hi + lo (hi = e4m3(x), lo = e4m3(x - hi)) and ships 3
    pre-padded 58x58 planes per image: (hi shifted down 2 rows, hi, lo).
  - Per 8-row psum tile, the 9 conv taps become 7 DR matmuls:
      2x "corner pair": slots (hi_down2, hi) = taps (2,kw)&(0,kw), kw in {0,2}
         -> these 4 taps carry plain e4m3 quantization error;
      5x "corrected": slots (hi, lo) same tap with the same +-1 weight in
         both slots -> exact to e4m3(lo residual) precision.
    Output rel err 1.77e-2 (limit 2e-2), measured bit-exact vs numpy model.
  - rhs is a contiguous 464-wide span (8 padded rows x 58) so the matmul
    ifmap AP stays 3D ([p][2][464] - the ISA limit); tap alignment is done
    by a fixed psum dst offset (D=2); pad/wrap contributions land in psum
    columns 0,1 of each 58-block, which the drain copy skips.
  - Weight-stationary ordering: each tap's weights held across the 4
    images' matmuls so the 256-col DoubleRow LDWEIGHTS hides; 4 PSUM banks
    accumulate while 4 drain.
  - Output stored bf16 (halves store traffic), upcast to f32 on host.
"""

import time

import numpy as np
import ml_dtypes

import concourse.bass as bass
import concourse.bacc as bacc
import concourse.mybir as mybir
import concourse.tile as tile
from concourse.bass_utils import run_bass_kernel_spmd

N_CORES = 8
B, CI, H, W = 32, 128, 56, 56
CO = 256
KH = KW = 3
BPC = B // N_CORES          # images per core
HP, WP = H + 2, W + 2       # padded spatial
ROWS_PER_TILE = 8
N_ROW_TILES = H // ROWS_PER_TILE   # 7
NSPAN = ROWS_PER_TILE * WP         # 464 contiguous rhs span
NPS = NSPAN + 2                    # 466 psum cols (D=2 shift)
PLANE = HP * WP                    # 3364

F32 = mybir.dt.float32
BF16 = mybir.dt.bfloat16
F8 = mybir.dt.float8e4
E4NP = ml_dtypes.float8_e4m3

# Uncorrected vertical corner pairs per preset (taps (2,kw)&(0,kw)).
# 7mm: err 1.77e-2 | 8mm: 1.26e-2 | 9mm: 1.8e-3  (offline, bf16 out)
PRESET_PAIRS = {"7mm": (0, 2), "8mm": (0,), "9mm": ()}

DEFAULT_CFG = dict(
    preset="7mm",
    img_bufs=2,                 # rotation depth per image tile
    out_bufs=8,
    psum_bufs=8,
    copy_engine="alternate",    # psum->sbuf drain engine
    store_queue="split",        # DMA queue for output stores: sync|scalar|split
    load_queue="sync",          # DMA queue for image loads
    weight_mode="dr",           # dr | swi (SW-interleaved weights)
    group8=False,               # 8-wide weight-stationary groups (2 row tiles)
    skip_store=False,           # ablation: only store on first pass
    load_once=False,            # ablation: load images once outside rep loop
)


def _mm_descs(preset: str):
    """Per row-tile MM list: (wa_tap, wb_tap, slot0, kh_base, kw_base).
    rhs base = (r0 + kh_base)*WP + kw_base, slots [slot0:slot0+2], dst D=2.
    slot planes: 0 = hi shifted down 2 rows, 1 = hi, 2 = lo."""
    pair_kws = PRESET_PAIRS[preset]
    descs = []
    uncorrected = set()
    for kw in pair_kws:
        descs.append((2 * KW + kw, 0 * KW + kw, 0, 0, kw))
        uncorrected.update((2 * KW + kw, 0 * KW + kw))
    for k9 in range(KH * KW):
        if k9 in uncorrected:
            continue
        kh, kw = divmod(k9, KW)
        descs.append((k9, k9, 1, kh, kw))
    return descs


def _load_imgs(nc, img_pool, x3_d, cfg):
    q = {"sync": nc.sync, "scalar": nc.scalar, "vector": nc.vector}[cfg["load_queue"]]
    imgs = []
    for n in range(BPC):
        xi = img_pool.tile([CI, 3, PLANE], F8, name=f"img{n}")
        q.dma_start(xi[:], x3_d[n])
        imgs.append(xi)
    return imgs


def _emit_body(nc, img_pool, out_pool, psum_pool, x3_d, y_d, w_sb, cfg,
               state, imgs=None):
    """One full pass: conv of the core's BPC images."""
    descs = _mm_descs(cfg["preset"])
    if cfg["weight_mode"] == "swi":
        perf = mybir.MatmulPerfMode.DoubleRowSwInterleave
    else:
        perf = mybir.MatmulPerfMode.DoubleRow
    if imgs is None:
        imgs = _load_imgs(nc, img_pool, x3_d, cfg)
    pass_i = state["pass"]
    state["pass"] += 1
    copy_i = 0
    if cfg["group8"]:
        tgroups = [(0, 1), (2, 3), (4, 5), (6,)]
    else:
        tgroups = [(t,) for t in range(N_ROW_TILES)]
    for tg in tgroups:
        for h in range(CO // 128):
            pss = {(t, n): psum_pool.tile([128, NPS], F32, name="ps")
                   for t in tg for n in range(BPC)}
            for m, (_, _, s0, khb, kwb) in enumerate(descs):
                if cfg["weight_mode"] == "swi":
                    # dim2 holds the co-half; 256 interleaved+reversed values
                    lhsT = w_sb[:, m, h, :].rearrange("p (c two) -> p two c", two=2)
                else:
                    lhsT = w_sb[:, m, :, h * 128:(h + 1) * 128]
                for t in tg:
                    b = (t * ROWS_PER_TILE + khb) * WP + kwb
                    # tail clip: overhang past the plane -> junk psum cols only
                    span = min(NSPAN, PLANE - b)
                    for n in range(BPC):
                        nc.tensor.matmul(
                            pss[t, n][:, 2:2 + span], lhsT,
                            imgs[n][:, s0:s0 + 2, b:b + span],
                            start=(m == 0), stop=(m == len(descs) - 1),
                            perf_mode=perf,
                        )
            for t in tg:
                r0 = t * ROWS_PER_TILE
                for n in range(BPC):
                    ob = out_pool.tile([128, ROWS_PER_TILE, W], BF16, name="ob")
                    psv = pss[t, n][:, 2:2 + NSPAN].rearrange(
                        "p (r c) -> p r c", c=WP)
                    ce = cfg["copy_engine"]
                    if ce == "alternate":
                        ce = "vector" if copy_i % 2 == 0 else "scalar"
                    if ce == "vector":
                        nc.vector.tensor_copy(ob[:], psv[:, :, 0:W])
                    else:
                        nc.scalar.copy(ob[:], psv[:, :, 0:W])
                    if not (cfg["skip_store"] and pass_i > 0):
                        sq = cfg["store_queue"]
                        if sq == "split":
                            sq = "sync" if copy_i % 2 == 0 else "scalar"
                        q = {"sync": nc.sync, "scalar": nc.scalar,
                             "vector": nc.vector}[sq]
                        q.dma_start(
                            y_d[n, h * 128:(h + 1) * 128,
                                r0: r0 + ROWS_PER_TILE, :],
                            ob[:],
                        )
                    copy_i += 1


def build_program(static_reps: int = 1, **overrides) -> bass.Bass:
    cfg = dict(DEFAULT_CFG, **overrides)
    n_mm = len(_mm_descs(cfg["preset"]))
    nc = bacc.Bacc(name="binconv2d")
    x3_d = nc.dram_tensor("x3", (BPC, CI, 3, PLANE), F8, kind="ExternalInput")
    w_d = nc.dram_tensor("w", (n_mm, CI, 2, CO), F8, kind="ExternalInput")
    y_d = nc.dram_tensor("y", (BPC, CO, H, W), BF16, kind="ExternalOutput")

    with tile.TileContext(nc) as tc:
        with (
            tc.tile_pool(name="wpool", bufs=1) as wpool,
            tc.tile_pool(name="imgs", bufs=cfg["img_bufs"]) as img_pool,
            tc.tile_pool(name="outb", bufs=cfg["out_bufs"]) as out_pool,
            tc.tile_pool(name="psum", bufs=cfg["psum_bufs"], space="PSUM") as psum_pool,
        ):
            w_sb = wpool.tile([CI, n_mm, 2, CO], F8)
            nc.sync.dma_start(w_sb[:], w_d[:].rearrange("m p s c -> p m s c"))
            state = {"pass": 0}
            imgs = None
            if cfg["load_once"]:
                imgs = _load_imgs(nc, img_pool, x3_d, cfg)
            for _ in range(static_reps):
                _emit_body(nc, img_pool, out_pool, psum_pool, x3_d, y_d, w_sb,
                           cfg, state, imgs)

    nc.finalize()
    return nc


def prep_weights(weights: np.ndarray, preset: str = None,
                 weight_mode: str = None) -> np.ndarray:
    """sign(weights) as e4m3 laid out per-MM: [n_mm, ci, slot=2, co].
    swi mode: [n_mm, ci, co_half, 256] interleaved+reversed pairs."""
    preset = preset or DEFAULT_CFG["preset"]
    weight_mode = weight_mode or DEFAULT_CFG["weight_mode"]
    bw = np.sign(np.asarray(weights, dtype=np.float32))
    # (co, ci, kh, kw) -> (kh*kw, ci, co)
    bw = np.ascontiguousarray(bw.transpose(2, 3, 1, 0)).reshape(KH * KW, CI, CO)
    descs = _mm_descs(preset)
    w8 = np.zeros((len(descs), CI, 2, CO), dtype=E4NP)
    for m, (a, b, s0, khb, kwb) in enumerate(descs):
        w8[m, :, 0, :] = bw[a].astype(E4NP)
        w8[m, :, 1, :] = bw[b].astype(E4NP)
    if weight_mode == "swi":
        wsw = np.zeros((len(descs), CI, 2, CO), dtype=E4NP)
        for h in range(CO // 128):
            half = w8[:, :, :, h * 128:(h + 1) * 128]     # [m, ci, 2, 128]
            wsw[:, :, h, 0::2] = half[:, :, 0, ::-1]
            wsw[:, :, h, 1::2] = half[:, :, 1, ::-1]
        w8 = wsw
    return w8


def prep_x(x: np.ndarray) -> np.ndarray:
    """x -> (B, CI, plane=3, 58*58) e4m3: (hi down-shifted 2 rows, hi, lo)."""
    x = np.asarray(x, dtype=np.float32)
    hi = x.astype(E4NP)
    lo = (x - hi.astype(np.float32)).astype(E4NP)
    xp = np.zeros((B, CI, 3, HP, WP), dtype=E4NP)
    xp[:, :, 1, 1:H + 1, 1:W + 1] = hi
    xp[:, :, 2, 1:H + 1, 1:W + 1] = lo
    xp[:, :, 0, 0:HP - 2, :] = xp[:, :, 1, 2:HP, :]
    return xp.reshape(B, CI, 3, PLANE)


def make_in_maps(x: np.ndarray, weights: np.ndarray, preset: str = None,
                 weight_mode: str = None) -> list[dict]:
    xp = prep_x(x)
    w8 = prep_weights(weights, preset, weight_mode)
    return [
        {"x3": xp[i * BPC:(i + 1) * BPC], "w": w8}
        for i in range(N_CORES)
    ]


def _run_once(x, weights) -> np.ndarray:
    nc = build_program()
    in_maps = make_in_maps(x, weights)
    res = run_bass_kernel_spmd(nc, in_maps, core_ids=list(range(N_CORES)))
    yb = np.concatenate([r["y"] for r in res.results], axis=0)
    return yb.astype(np.float32)


_SUBPROC_SRC = """
import sys, numpy as np
sys.path.insert(0, sys.argv[1])
import kernel as K
x = np.load(sys.argv[2]); w = np.load(sys.argv[3])
np.save(sys.argv[4], K._run_once(x, w))
"""


def kernel(x, weights) -> np.ndarray:
    x = np.ascontiguousarray(np.asarray(x, dtype=np.float32))
    weights = np.ascontiguousarray(np.asarray(weights, dtype=np.float32))
    try:
        return _run_once(x, weights)
    except Exception as first_exc:
        # Transient device wedges (NRT_EXEC_UNIT_UNRECOVERABLE, mesh desync)
        # poison the in-process PJRT client; only a fresh process recovers.
        import os
        import subprocess
        import sys
        import tempfile

        last_exc = first_exc
        moddir = os.path.dirname(os.path.abspath(__file__))
        for attempt in range(2):
            time.sleep(10 * (attempt + 1))
            try:
                with tempfile.TemporaryDirectory() as td:
                    xp, wp, yp = (os.path.join(td, f) for f in
                                  ("x.npy", "w.npy", "y.npy"))
                    np.save(xp, x)
                    np.save(wp, weights)
                    subprocess.run(
                        [sys.executable, "-c", _SUBPROC_SRC, moddir, xp, wp, yp],
                        check=True, timeout=900,
                    )
                    return np.load(yp)
            except Exception as e:
                last_exc = e
        raise last_exc


# revision 14
# speedup vs baseline: 3.7323x; 1.1066x over previous
"""Binary-weight 3x3 conv2d (stride 1, pad 1) on 8 TRN2 NeuronCores.

Reference computes y = conv2d(x, sign(weights)) in NCHW/OIHW, f32.
  x: (32, 128, 56, 56) f32, weights: (256, 128, 3, 3) f32 -> y: (32, 256, 56, 56) f32

Strategy (data-parallel on batch: 8 cores x 4 images; weights replicated):
  - All compute in fp8 e4m3 DoubleRow matmuls (2 multiplies/cell/cycle).
    Host splits x = hi + lo (hi = e4m3(x), lo = e4m3(x - hi)) and ships 3
    pre-padded 58x58 planes per image: (hi shifted down 2 rows, hi, lo).
  - Per 8-row psum tile, the 9 conv taps become 7 DR matmuls:
      2x "corner pair": slots (hi_down2, hi) = taps (2,kw)&(0,kw), kw in {0,2}
         -> these 4 taps carry plain e4m3 quantization error;
      5x "corrected": slots (hi, lo) same tap with the same +-1 weight in
         both slots -> exact to e4m3(lo residual) precision.
    Output rel err 1.77e-2 (limit 2e-2), measured bit-exact vs numpy model.
  - rhs is a contiguous 464-wide span (8 padded rows x 58) so the matmul
    ifmap AP stays 3D ([p][2][464] - the ISA limit); tap alignment is done
    by a fixed psum dst offset (D=2); pad/wrap contributions land in psum
    columns 0,1 of each 58-block, which the drain copy skips.
  - Weight-stationary ordering: each tap's weights held across the 4
    images' matmuls so the 256-col DoubleRow LDWEIGHTS hides; 4 PSUM banks
    accumulate while 4 drain. Stores split across both HWDGE queues.
  - Output stored bf16 (halves store traffic), upcast to f32 on host.

Measured: ~51-55us/pass (burst-slope estimator; bf16 9-matmul baseline was
111-114us at the same measurement = 2.1x). Rel err 1.767e-2 (gate 2e-2),
identical to the numpy model of the scheme. Rejected along the way: e3m4
DoubleRow (walrus INTERNAL_ERROR - compiler, not HW, blocks it), SwInterleave
weights (+3us), 8-wide weight groups (+6us), fp8 without DoubleRow (no gain:
runs at bf16 speed), 4D strided rhs APs (3x slowdown - matmul ifmap APs
must stay 3D, hence the contiguous 464-span + psum dst-shift trick).
"""

import time

import numpy as np
import ml_dtypes

import concourse.bass as bass
import concourse.bacc as bacc
import concourse.mybir as mybir
import concourse.tile as tile
from concourse.bass_utils import run_bass_kernel_spmd

N_CORES = 8
B, CI, H, W = 32, 128, 56, 56
CO = 256
KH = KW = 3
BPC = B // N_CORES          # images per core
HP, WP = H + 2, W + 2       # padded spatial
ROWS_PER_TILE = 8
N_ROW_TILES = H // ROWS_PER_TILE   # 7
NSPAN = ROWS_PER_TILE * WP         # 464 contiguous rhs span
NPS = NSPAN + 2                    # 466 psum cols (D=2 shift)
PLANE = HP * WP                    # 3364

F32 = mybir.dt.float32
BF16 = mybir.dt.bfloat16
F8 = mybir.dt.float8e4
E4NP = ml_dtypes.float8_e4m3

# Uncorrected vertical corner pairs per preset (taps (2,kw)&(0,kw)).
# 7mm: err 1.77e-2 | 8mm: 1.26e-2 | 9mm: 1.8e-3  (offline, bf16 out)
PRESET_PAIRS = {"7mm": (0, 2), "8mm": (0,), "9mm": ()}

DEFAULT_CFG = dict(
    preset="7mm",
    img_bufs=2,                 # rotation depth per image tile
    out_bufs=8,
    psum_bufs=8,
    copy_engine="alternate",    # psum->sbuf drain engine
    store_queue="split",        # DMA queue for output stores: sync|scalar|split
    load_queue="sync",          # DMA queue for image loads
    weight_mode="dr",           # dr | swi (SW-interleaved weights)
    group8=False,               # 8-wide weight-stationary groups (2 row tiles)
    skip_store=False,           # ablation: only store on first pass
    load_once=False,            # ablation: load images once outside rep loop
)


def _mm_descs(preset: str):
    """Per row-tile MM list: (wa_tap, wb_tap, slot0, kh_base, kw_base).
    rhs base = (r0 + kh_base)*WP + kw_base, slots [slot0:slot0+2], dst D=2.
    slot planes: 0 = hi shifted down 2 rows, 1 = hi, 2 = lo."""
    pair_kws = PRESET_PAIRS[preset]
    descs = []
    uncorrected = set()
    for kw in pair_kws:
        descs.append((2 * KW + kw, 0 * KW + kw, 0, 0, kw))
        uncorrected.update((2 * KW + kw, 0 * KW + kw))
    for k9 in range(KH * KW):
        if k9 in uncorrected:
            continue
        kh, kw = divmod(k9, KW)
        descs.append((k9, k9, 1, kh, kw))
    return descs


def _load_imgs(nc, img_pool, x3_d, cfg):
    q = {"sync": nc.sync, "scalar": nc.scalar, "vector": nc.vector}[cfg["load_queue"]]
    imgs = []
    for n in range(BPC):
        xi = img_pool.tile([CI, 3, PLANE], F8, name=f"img{n}")
        q.dma_start(xi[:], x3_d[n])
        imgs.append(xi)
    return imgs


def _emit_body(nc, img_pool, out_pool, psum_pool, x3_d, y_d, w_sb, cfg,
               state, imgs=None):
    """One full pass: conv of the core's BPC images."""
    descs = _mm_descs(cfg["preset"])
    if cfg["weight_mode"] == "swi":
        perf = mybir.MatmulPerfMode.DoubleRowSwInterleave
    else:
        perf = mybir.MatmulPerfMode.DoubleRow
    if imgs is None:
        imgs = _load_imgs(nc, img_pool, x3_d, cfg)
    pass_i = state["pass"]
    state["pass"] += 1
    copy_i = 0
    if cfg["group8"]:
        tgroups = [(0, 1), (2, 3), (4, 5), (6,)]
    else:
        tgroups = [(t,) for t in range(N_ROW_TILES)]
    for tg in tgroups:
        for h in range(CO // 128):
            pss = {(t, n): psum_pool.tile([128, NPS], F32, name="ps")
                   for t in tg for n in range(BPC)}
            for m, (_, _, s0, khb, kwb) in enumerate(descs):
                if cfg["weight_mode"] == "swi":
                    # dim2 holds the co-half; 256 interleaved+reversed values
                    lhsT = w_sb[:, m, h, :].rearrange("p (c two) -> p two c", two=2)
                else:
                    lhsT = w_sb[:, m, :, h * 128:(h + 1) * 128]
                for t in tg:
                    b = (t * ROWS_PER_TILE + khb) * WP + kwb
                    # tail clip: overhang past the plane -> junk psum cols only
                    span = min(NSPAN, PLANE - b)
                    for n in range(BPC):
                        nc.tensor.matmul(
                            pss[t, n][:, 2:2 + span], lhsT,
                            imgs[n][:, s0:s0 + 2, b:b + span],
                            start=(m == 0), stop=(m == len(descs) - 1),
                            perf_mode=perf,
                        )
            for t in tg:
                r0 = t * ROWS_PER_TILE
                for n in range(BPC):
                    ob = out_pool.tile([128, ROWS_PER_TILE, W], BF16, name="ob")
                    psv = pss[t, n][:, 2:2 + NSPAN].rearrange(
                        "p (r c) -> p r c", c=WP)
                    ce = cfg["copy_engine"]
                    if ce == "alternate":
                        ce = "vector" if copy_i % 2 == 0 else "scalar"
                    if ce == "vector":
                        nc.vector.tensor_copy(ob[:], psv[:, :, 0:W])
                    else:
                        nc.scalar.copy(ob[:], psv[:, :, 0:W])
                    if not (cfg["skip_store"] and pass_i > 0):
                        sq = cfg["store_queue"]
                        if sq == "split":
                            sq = "sync" if copy_i % 2 == 0 else "scalar"
                        q = {"sync": nc.sync, "scalar": nc.scalar,
                             "vector": nc.vector}[sq]
                        q.dma_start(
                            y_d[n, h * 128:(h + 1) * 128,
                                r0: r0 + ROWS_PER_TILE, :],
                            ob[:],
                        )
                    copy_i += 1


def build_program(static_reps: int = 1, **overrides) -> bass.Bass:
    cfg = dict(DEFAULT_CFG, **overrides)
    n_mm = len(_mm_descs(cfg["preset"]))
    nc = bacc.Bacc(name="binconv2d")
    x3_d = nc.dram_tensor("x3", (BPC, CI, 3, PLANE), F8, kind="ExternalInput")
    w_d = nc.dram_tensor("w", (n_mm, CI, 2, CO), F8, kind="ExternalInput")
    y_d = nc.dram_tensor("y", (BPC, CO, H, W), BF16, kind="ExternalOutput")

    with tile.TileContext(nc) as tc:
        with (
            tc.tile_pool(name="wpool", bufs=1) as wpool,
            tc.tile_pool(name="imgs", bufs=cfg["img_bufs"]) as img_pool,
            tc.tile_pool(name="outb", bufs=cfg["out_bufs"]) as out_pool,
            tc.tile_pool(name="psum", bufs=cfg["psum_bufs"], space="PSUM") as psum_pool,
        ):
            w_sb = wpool.tile([CI, n_mm, 2, CO], F8)
            nc.sync.dma_start(w_sb[:], w_d[:].rearrange("m p s c -> p m s c"))
            state = {"pass": 0}
            imgs = None
            if cfg["load_once"]:
                imgs = _load_imgs(nc, img_pool, x3_d, cfg)
            for _ in range(static_reps):
                _emit_body(nc, img_pool, out_pool, psum_pool, x3_d, y_d, w_sb,
                           cfg, state, imgs)

    nc.finalize()
    return nc


def prep_weights(weights: np.ndarray, preset: str = None,
                 weight_mode: str = None) -> np.ndarray:
    """sign(weights) as e4m3 laid out per-MM: [n_mm, ci, slot=2, co].
    swi mode: [n_mm, ci, co_half, 256] interleaved+reversed pairs."""
    preset = preset or DEFAULT_CFG["preset"]
    weight_mode = weight_mode or DEFAULT_CFG["weight_mode"]
    bw = np.sign(np.asarray(weights, dtype=np.float32))
    # (co, ci, kh, kw) -> (kh*kw, ci, co)
    bw = np.ascontiguousarray(bw.transpose(2, 3, 1, 0)).reshape(KH * KW, CI, CO)
    descs = _mm_descs(preset)
    w8 = np.zeros((len(descs), CI, 2, CO), dtype=E4NP)
    for m, (a, b, s0, khb, kwb) in enumerate(descs):
        w8[m, :, 0, :] = bw[a].astype(E4NP)
        w8[m, :, 1, :] = bw[b].astype(E4NP)
    if weight_mode == "swi":
        wsw = np.zeros((len(descs), CI, 2, CO), dtype=E4NP)
        for h in range(CO // 128):
            half = w8[:, :, :, h * 128:(h + 1) * 128]     # [m, ci, 2, 128]
            wsw[:, :, h, 0::2] = half[:, :, 0, ::-1]
            wsw[:, :, h, 1::2] = half[:, :, 1, ::-1]
        w8 = wsw
    return w8


def prep_x(x: np.ndarray) -> np.ndarray:
    """x -> (B, CI, plane=3, 58*58) e4m3: (hi down-shifted 2 rows, hi, lo)."""
    x = np.asarray(x, dtype=np.float32)
    hi = x.astype(E4NP)
    lo = (x - hi.astype(np.float32)).astype(E4NP)
    xp = np.zeros((B, CI, 3, HP, WP), dtype=E4NP)
    xp[:, :, 1, 1:H + 1, 1:W + 1] = hi
    xp[:, :, 2, 1:H + 1, 1:W + 1] = lo
    xp[:, :, 0, 0:HP - 2, :] = xp[:, :, 1, 2:HP, :]
    return xp.reshape(B, CI, 3, PLANE)


def make_in_maps(x: np.ndarray, weights: np.ndarray, preset: str = None,
                 weight_mode: str = None) -> list[dict]:
    xp = prep_x(x)
    w8 = prep_weights(weights, preset, weight_mode)
    return [
        {"x3": xp[i * BPC:(i + 1) * BPC], "w": w8}
        for i in range(N_CORES)
    ]


def _run_once(x, weights) -> np.ndarray:
    nc = build_program()
    in_maps = make_in_maps(x, weights)
    res = run_bass_kernel_spmd(nc, in_maps, core_ids=list(range(N_CORES)))
    yb = np.concatenate([r["y"] for r in res.results], axis=0)
    return yb.astype(np.float32)


_SUBPROC_SRC = """
import sys, numpy as np
sys.path.insert(0, sys.argv[1])
import kernel as K
x = np.load(sys.argv[2]); w = np.load(sys.argv[3])
np.save(sys.argv[4], K._run_once(x, w))
"""


def kernel(x, weights) -> np.ndarray:
    x = np.ascontiguousarray(np.asarray(x, dtype=np.float32))
    weights = np.ascontiguousarray(np.asarray(weights, dtype=np.float32))
    try:
        return _run_once(x, weights)
    except Exception as first_exc:
        # Transient device wedges (NRT_EXEC_UNIT_UNRECOVERABLE, mesh desync)
        # poison the in-process PJRT client; only a fresh process recovers.
        import os
        import subprocess
        import sys
        import tempfile

        last_exc = first_exc
        moddir = os.path.dirname(os.path.abspath(__file__))
        for attempt in range(2):
            time.sleep(10 * (attempt + 1))
            try:
                with tempfile.TemporaryDirectory() as td:
                    xp, wp, yp = (os.path.join(td, f) for f in
                                  ("x.npy", "w.npy", "y.npy"))
                    np.save(xp, x)
                    np.save(wp, weights)
                    subprocess.run(
                        [sys.executable, "-c", _SUBPROC_SRC, moddir, xp, wp, yp],
                        check=True, timeout=900,
                    )
                    return np.load(yp)
            except Exception as e:
                last_exc = e
        raise last_exc
